# revision 1
# baseline (speedup 1.0000x reference)
"""Conv2d 3x3 (N=32, C_in=128, H=W=56, C_out=256, stride 1, pad 1) on 8 trn2
NeuronCores.

Strategy: data-parallel over batch N (4 images per core, weight/bias
replicated).  Per core the conv runs as an implicit GEMM: C_in=128 sits
exactly on the 128 SBUF partitions (the contraction dim), and for each of
the 9 kernel taps one matmul of lhsT=[C_in, C_out_half] against a shifted
window of the zero-padded input accumulates into PSUM.  fp32 data is
bit-cast to float32r so the PE streams at 1 cycle/row (4x over fp32).
Spatial tiling: 8 output rows (8*56=448 <= 512 fp32 PSUM bank) per matmul
group; bias is fused into the PSUM->SBUF evacuation on the vector engine.
"""

import sys

if "/opt/trn_rl_repo" not in sys.path:
    sys.path.insert(0, "/opt/trn_rl_repo")

import numpy as np

N, C_IN, H, W = 32, 128, 56, 56
C_OUT, KH, KW = 256, 3, 3
N_CORES = 8
IMGS_PER_CORE = N // N_CORES  # 4
HP, WP = H + 2, W + 2  # zero-padded input
ROWS_PER_TILE = 8
N_TILES = H // ROWS_PER_TILE  # 7
TILE_FREE = ROWS_PER_TILE * W  # 448 fp32 <= 512 (one PSUM bank)
N_HALF = C_OUT // 128  # 2

_CACHE = {}


def _build_program():
    import concourse.mybir as mybir
    import concourse.tile as tile
    from concourse import bacc

    F32 = mybir.dt.float32
    F32R = mybir.dt.float32r

    nc = bacc.Bacc("TRN2", target_bir_lowering=False, debug=False,
                   enable_asserts=False)

    xp = nc.dram_tensor("xp", [IMGS_PER_CORE, C_IN, HP, WP], F32R,
                        kind="ExternalInput").ap()
    w = nc.dram_tensor("w", [C_IN, KH * KW, C_OUT], F32R,
                       kind="ExternalInput").ap()
    b = nc.dram_tensor("b", [128, N_HALF], F32, kind="ExternalInput").ap()
    out = nc.dram_tensor("out", [IMGS_PER_CORE, C_OUT, H, W], F32,
                         kind="ExternalOutput").ap()
    out_v = out.rearrange("n c a b -> n c (a b)")

    with tile.TileContext(nc) as tc:
        with (
            tc.tile_pool(name="consts", bufs=1) as consts,
            tc.tile_pool(name="xin", bufs=2) as xin,
            tc.tile_pool(name="outp", bufs=3) as outp,
            tc.tile_pool(name="psum", bufs=8, space="PSUM") as psum,
        ):
            w_sb = consts.tile([C_IN, KH * KW, C_OUT], F32R)
            nc.sync.dma_start(out=w_sb[:], in_=w)
            b_sb = consts.tile([128, N_HALF], F32)
            nc.sync.dma_start(out=b_sb[:], in_=b)

            for img in range(IMGS_PER_CORE):
                xt = xin.tile([C_IN, HP, WP], F32R)
                nc.sync.dma_start(out=xt[:], in_=xp[img])
                for h in range(N_HALF):
                    ot = outp.tile([128, H * W], F32)
                    for t in range(N_TILES):
                        pt = psum.tile([128, TILE_FREE], F32)
                        for k in range(KH * KW):
                            kh, kw = divmod(k, KW)
                            r0 = ROWS_PER_TILE * t + kh
                            nc.tensor.matmul(
                                pt[:, :],
                                lhsT=w_sb[:, k, h * 128:(h + 1) * 128],
                                rhs=xt[:, r0:r0 + ROWS_PER_TILE, kw:kw + W],
                                start=(k == 0),
                                stop=(k == KH * KW - 1),
                            )
                        nc.vector.tensor_scalar_add(
                            out=ot[:, t * TILE_FREE:(t + 1) * TILE_FREE],
                            in0=pt[:, :],
                            scalar1=b_sb[:, h:h + 1],
                        )
                    nc.sync.dma_start(out=out_v[img, h * 128:(h + 1) * 128],
                                      in_=ot[:])
    nc.compile()
    return nc


def get_program():
    if "nc" not in _CACHE:
        _CACHE["nc"] = _build_program()
    return _CACHE["nc"]


def make_in_maps(x, weight, bias):
    """Host-side prep: zero-pad x, retile weight/bias, shard over cores."""
    x = np.asarray(x, dtype=np.float32)
    weight = np.asarray(weight, dtype=np.float32)
    bias = np.asarray(bias, dtype=np.float32)

    xpad = np.zeros((N, C_IN, HP, WP), dtype=np.float32)
    xpad[:, :, 1:1 + H, 1:1 + W] = x
    # w[ci, k, co] = weight[co, ci, kh, kw]
    w_t = np.ascontiguousarray(
        weight.transpose(1, 2, 3, 0).reshape(C_IN, KH * KW, C_OUT))
    # b2[p, h] = bias[h*128 + p]
    b2 = np.ascontiguousarray(bias.reshape(N_HALF, 128).T)

    return [
        {
            "xp": np.ascontiguousarray(
                xpad[i * IMGS_PER_CORE:(i + 1) * IMGS_PER_CORE]),
            "w": w_t,
            "b": b2,
        }
        for i in range(N_CORES)
    ]


def kernel(x, weight, bias):
    from concourse.bass_utils import run_bass_kernel_spmd

    nc = get_program()
    in_maps = make_in_maps(x, weight, bias)
    res = run_bass_kernel_spmd(nc, in_maps, core_ids=list(range(N_CORES)))
    return np.concatenate([res.results[i]["out"] for i in range(N_CORES)],
                          axis=0)
